# revision 9
# baseline (speedup 1.0000x reference)
"""Trainium2 Bass kernel for nn_CrossAttention3D (B=4, C=D=512, H=W=64).

Strategy
--------
reference:  x=(b,c,s) with s=h*w=4096;  Q/K/V = per-pixel linear (1x1 conv),
            sim = Q K^T * D^-0.5, attn = softmax(sim), o = attn V,
            y = o Wo^T + bo.

Sharding: 8 cores = (batch b in 0..3) x (query-half in 0..1); each core does
attention + output for its 2048 query tokens. No collectives.

Algebraic refactor (host folds weight-weight products, exact math):
  sim[t,s] = K_t . Q_s = x_t^T H xq_s + x_t^T wt + c_s
      H  = Wk^T (Wq*scale)   [c,c]
      wt = Wk^T (bq*scale)   [c]     (rides as the U bias)
      c_s depends only on s -> cancels in softmax.
  U = H xq + wt                      (replaces Q AND K projections)
  P = exp(x^T U)
  Z = sum_t x_t P[t,s]
  y = W2 Z / l + bo'                 (replaces V proj AND out proj)
      W2 = Wo Wv,  bo' = Wo bv + bo
      l  = ones^T P

FP8 (e4m3) + DoubleRow: every matmul contracts TWO 128-deep k-chunks per
instruction (lhsT [128,2,M], rhs [128,2,N]) at 0.5 cycles/row -- 2-4x the
bf16 rate.  Probe-validated scale factors keep all fp8 operands in e4m3's
normal range:
  x8  = fp8(x)                       (std 1.0)
  ht  = fp8(256*H^T), U psum = 256*(H xq); u8 = fp8(psum/8 + 32*wt) = 32*U
  sim psum = 32*sim;  P8 = exp(psum/32)   (ACT scale)
  z8  = fp8(Z/16),  w2t = fp8(16*W2^T) -> py = W2 Z exactly
Emulated end-to-end rel err 9.3e-3 (gate 2e-2; bf16 version was 5.8e-4).

The sim/Z loop is software-pipelined at t-chunk-PAIR granularity (sim runs
AHEAD of Z by 2 pairs, crossing query-tile boundaries).  The denominator l
accumulates as ONE bf16 DVE add per pair ([128,1024], 2x DVE rate); the last
pair joins the cross-partition ones-matmul reduction directly so the PE never
waits on the DVE tail.  U bias + Z evacuation ride on DVE tensor_scalar,
keeping ACT free for the 64 exp instructions that are its floor.
"""

import numpy as np
import ml_dtypes

bf16 = ml_dtypes.bfloat16
f8np = ml_dtypes.float8_e4m3

# Problem constants (hardcoded per harness contract)
B, C, H, W = 4, 512, 64, 64
D = 512
S = H * W          # 4096 tokens per batch
NCORES = 8
SQ = S * B // NCORES  # 2048 query tokens per core
P = 128            # partitions
NC_C = C // P      # 4 c-chunks
NT = S // P        # 32 t-chunks (keys)
NPAIR = NT // 2    # 16 t-chunk pairs
NSQ = SQ // 512    # 4 query tiles of 512
NTT = S // 512     # 8 t-tiles of 512
AHEAD = 4          # sim-ahead-of-Z pipeline depth, in pairs
NWARM = 12         # PE warm-up matmuls during the DMA head

GH = 256.0         # host scale on H (fp8 range)
GU = 32.0          # scale on U in fp8
GW = 16.0          # host scale on W2
GZI = 1.0 / 16.0   # scale applied to Z at evacuation (cancels GW)


def build_bass():
    """Build the single-core SPMD Bass program."""
    import concourse.mybir as mybir
    import concourse.tile as tile
    from concourse import bacc

    fp32 = mybir.dt.float32
    bfl = mybir.dt.bfloat16
    f8 = mybir.dt.float8e4
    AF = mybir.ActivationFunctionType
    DR = mybir.MatmulPerfMode.DoubleRow
    MUL = mybir.AluOpType.mult
    ADD = mybir.AluOpType.add

    nc = bacc.Bacc("TRN2", target_bir_lowering=False)

    # x is shipped per-core with the core's own query-half chunks FIRST
    # (chunk order rotated on host), so x_t[0..3] double as xq tiles.
    x_d = nc.dram_tensor("x", (C, S), f8, kind="ExternalInput")
    xt_d = nc.dram_tensor("xt", (S, C), f8, kind="ExternalInput")
    ht_d = nc.dram_tensor("ht", (C, C), f8, kind="ExternalInput")
    w2t_d = nc.dram_tensor("w2t", (C, C), f8, kind="ExternalInput")
    wt_d = nc.dram_tensor("wt", (P, NC_C), fp32, kind="ExternalInput")
    bop_d = nc.dram_tensor("bop", (P, NC_C), fp32, kind="ExternalInput")
    y_d = nc.dram_tensor("y", (C, SQ), fp32, kind="ExternalOutput")

    with tile.TileContext(nc) as tc:
        with (
            tc.tile_pool(name="const", bufs=1) as const,
            tc.tile_pool(name="pt", bufs=6) as ptp,
            tc.tile_pool(name="zsb", bufs=2) as zsb,
            tc.tile_pool(name="ysb", bufs=3) as ysb,
            tc.tile_pool(name="small", bufs=2) as small,
            tc.tile_pool(name="ps", bufs=3, space="PSUM") as ps,
            tc.tile_pool(name="pso", bufs=1, space="PSUM") as pso,
            tc.tile_pool(name="psl", bufs=1, space="PSUM") as psl,
        ):
            # ---- PE warm-up burst: no input deps, runs while DMAs land ----
            wtile = const.tile([P, 2, 512], f8)
            nc.vector.memset(wtile, 0.01)
            wps = ps.tile([P, 512], fp32, tag="ps")
            for i in range(NWARM):
                nc.tensor.matmul(wps, wtile[:, :, 0:P], wtile,
                                 start=(i == 0), stop=(i == NWARM - 1),
                                 perf_mode=DR)
            wdump = small.tile([P, 16], fp32, tag="wdump")
            nc.vector.tensor_copy(wdump, wps[:, 0:16])

            # ---- loads, ordered + chunked so U-projection starts ASAP ----
            ht_sb = const.tile([P, NC_C, C], f8)
            nc.sync.dma_start(ht_sb, ht_d[:].rearrange("(o p) c -> p o c", p=P))
            wt_sb = const.tile([P, NC_C], fp32)
            nc.sync.dma_start(wt_sb, wt_d[:])
            x_t = [const.tile([P, NC_C, 512], f8, tag=f"x{tt}", name=f"x{tt}")
                   for tt in range(NTT)]
            xt_sb = const.tile([P, NT, C], f8)

            def load_x(tt):
                nc.sync.dma_start(
                    x_t[tt], x_d[:, tt * 512:(tt + 1) * 512]
                    .rearrange("(o p) s -> p o s", p=P))

            def load_xt(i):
                nc.sync.dma_start(
                    xt_sb[:, i * 8:(i + 1) * 8, :],
                    xt_d[i * 1024:(i + 1) * 1024, :]
                    .rearrange("(o p) c -> p o c", p=P))

            load_x(0); load_x(1); load_xt(0)
            load_x(2); load_x(3); load_xt(1)
            load_x(4); load_x(5); load_xt(2)
            load_x(6); load_x(7); load_xt(3)
            w2t_sb = const.tile([P, NC_C, C], f8)
            nc.sync.dma_start(w2t_sb, w2t_d[:].rearrange("(o p) c -> p o c", p=P))
            bop_sb = const.tile([P, NC_C], fp32)
            nc.sync.dma_start(bop_sb, bop_d[:])
            ones_bf = const.tile([P, 1], bfl)
            nc.vector.memset(ones_bf, 1.0)
            ones8 = const.tile([P, 1], f8)
            nc.vector.memset(ones8, 1.0)

            u_sb = const.tile([P, NC_C, SQ], f8)   # u8[c, sq] = 32*U

            # ---- U projection: u8 = (256*H @ xq)/8 + 32*wt  (DVE epilogue) ----
            for st in range(NSQ):
                for co in range(NC_C):
                    pu = ps.tile([P, 512], fp32, tag="ps")
                    for j in range(2):
                        nc.tensor.matmul(
                            pu,
                            ht_sb[:, 2 * j:2 * j + 2, co * P:(co + 1) * P],
                            x_t[st][:, 2 * j:2 * j + 2, :],
                            start=(j == 0), stop=(j == 1), perf_mode=DR,
                        )
                    nc.vector.tensor_scalar(
                        u_sb[:, co, st * 512:(st + 1) * 512], pu,
                        1.0 / 8.0, wt_sb[:, co:co + 1], MUL, ADD)

            # ---- attention: flat software pipeline over (st, pair) units ----
            units = [(st, pj) for st in range(NSQ) for pj in range(NPAIR)]
            total = len(units)
            pts = [None] * total
            state = {}  # per-st live tiles: po, l_acc

            def sim_step(i):
                st, pj = units[i]
                pt = ptp.tile([P, 2, 512], f8, tag="pt")
                for k in range(2):
                    tch = 2 * pj + k
                    pss = ps.tile([P, 512], fp32, tag="ps")
                    for j in range(2):
                        nc.tensor.matmul(
                            pss,
                            x_t[tch // 4][:, 2 * j:2 * j + 2,
                                          (tch % 4) * P:(tch % 4 + 1) * P],
                            u_sb[:, 2 * j:2 * j + 2,
                                 st * 512:(st + 1) * 512],
                            start=(j == 0), stop=(j == 1), perf_mode=DR,
                        )
                    nc.scalar.activation(pt[:, k, :], pss, AF.Exp,
                                         scale=1.0 / GU)
                pts[i] = pt

            for i in range(AHEAD):
                sim_step(i)
            for i, (st, pj) in enumerate(units):
                if i + AHEAD < total and pj != NPAIR - 1:
                    sim_step(i + AHEAD)
                pt = pts[i]
                if pj == 0:
                    state["po"] = pso.tile([P, NC_C, 512], fp32, tag="po",
                                           name="po")
                    state["lacc"] = small.tile([P, 2, 512], bfl, tag="lacc",
                                               name="lacc")
                po, l_acc = state["po"], state["lacc"]
                # Z[c, sq] += xt[t-pair, c-chunk]^T P  (fp8 DoubleRow)
                for cc in range(NC_C):
                    nc.tensor.matmul(
                        po[:, cc, :],
                        xt_sb[:, 2 * pj:2 * pj + 2, cc * P:(cc + 1) * P],
                        pt,
                        start=(pj == 0), stop=(pj == NPAIR - 1), perf_mode=DR,
                    )
                # the l partial-sum chain lives on the otherwise-idle GpSimd
                # so the DVE is free for the epilogue's evac/ytmp ops
                if pj == 0:
                    nc.gpsimd.tensor_copy(l_acc, pt)
                elif pj < NPAIR - 1:
                    # last pair skips the chain; it joins the denominator
                    # directly in PSUM via two fp8 matmuls so the PE never
                    # waits on the accumulator tail
                    nc.gpsimd.tensor_add(out=l_acc, in0=l_acc, in1=pt)
                    pts[i] = None

                if pj == NPAIR - 1:
                    # ---- epilogue for query tile st ----
                    # emitted BEFORE the look-ahead sim_step so the z
                    # evacuations sit ahead of the next exps in the ACT queue
                    sq_sl = slice(st * 512, (st + 1) * 512)
                    z_t = zsb.tile([P, NC_C, 512], f8, tag="z")
                    nc.vector.tensor_scalar_mul(z_t[:, 0, :], po[:, 0, :], GZI)
                    nc.scalar.activation(z_t[:, 1, :], po[:, 1, :],
                                         AF.Identity, scale=GZI)

                    pl = psl.tile([1, 512], fp32, tag="pl")
                    nc.tensor.matmul(pl, ones_bf, l_acc[:, 0, :],
                                     start=True, stop=False)
                    nc.tensor.matmul(pl, ones_bf, l_acc[:, 1, :],
                                     start=False, stop=False)
                    nc.tensor.matmul(pl, ones8, pt[:, 0, :],
                                     start=False, stop=False)
                    nc.tensor.matmul(pl, ones8, pt[:, 1, :],
                                     start=False, stop=True)
                    pts[i] = None
                    rl = small.tile([1, 512], fp32, tag="rl")
                    nc.vector.reciprocal_approx_fast(rl, pl)
                    rlb = small.tile([P, 512], fp32, tag="rlb")
                    nc.gpsimd.partition_broadcast(rlb, rl)
                    nc.vector.tensor_scalar_mul(z_t[:, 2, :], po[:, 2, :], GZI)
                    nc.scalar.activation(z_t[:, 3, :], po[:, 3, :],
                                         AF.Identity, scale=GZI)

                    # y[c, sq] = (W2 Z) * rl + bo'   (ci-pair outer: the first
                    # 4 matmuls need only z chunks 0-1, so they start right
                    # after the first evacuation ops land)
                    py = pso.tile([P, NC_C, 512], fp32, tag="po")
                    for j in range(2):
                        for co in range(NC_C):
                            nc.tensor.matmul(
                                py[:, co, :],
                                w2t_sb[:, 2 * j:2 * j + 2,
                                       co * P:(co + 1) * P],
                                z_t[:, 2 * j:2 * j + 2, :],
                                start=(j == 0), stop=(j == 1), perf_mode=DR,
                            )
                    if i + AHEAD < total:
                        sim_step(i + AHEAD)
                    # py -> y: GpSimd cannot read PSUM, so odd chunks drain
                    # via an ACT Identity copy to SBUF (then GpSimd applies
                    # rl); even chunks are direct DVE muls from PSUM.  Two
                    # engines drain py in parallel -> po banks free sooner.
                    for co in range(NC_C):
                        ytmp = ysb.tile([P, 512], fp32, tag="ytmp")
                        if co % 2 == 0:
                            nc.vector.tensor_mul(out=ytmp, in0=py[:, co, :],
                                                 in1=rlb)
                        else:
                            yc = ysb.tile([P, 512], fp32, tag="yc")
                            nc.scalar.activation(yc, py[:, co, :], AF.Identity)
                            nc.gpsimd.tensor_mul(out=ytmp, in0=yc, in1=rlb)
                        yt = ysb.tile([P, 512], fp32, tag="y")
                        nc.scalar.activation(yt, ytmp, AF.Identity,
                                             bias=bop_sb[:, co:co + 1])
                        nc.sync.dma_start(y_d[co * P:(co + 1) * P, sq_sl], yt)

    nc.finalize()
    return nc


def make_in_maps(q, Wq, bq, Wk, bk, Wv, bv, Wo, bo):
    """Host-side sharding + weight folding. Returns list of 8 input dicts."""
    scale = float(D) ** -0.5
    x_full = np.ascontiguousarray(q.reshape(B, C, S)).astype(np.float32)

    Hm = Wk.T.astype(np.float32) @ (Wq.astype(np.float32) * scale)   # [c, c]
    wt = Wk.T.astype(np.float32) @ (bq.astype(np.float32) * scale)   # [c]
    W2 = Wo.astype(np.float32) @ Wv.astype(np.float32)               # [c, c]
    bop = Wo.astype(np.float32) @ bv.astype(np.float32) + bo         # [c]

    ht = np.ascontiguousarray(Hm.T * GH).astype(f8np)
    w2t = np.ascontiguousarray(W2.T * GW).astype(f8np)
    wt_t = np.ascontiguousarray((GU * wt).reshape(NC_C, P).T).astype(np.float32)
    bop_t = np.ascontiguousarray(bop.reshape(NC_C, P).T).astype(np.float32)

    in_maps = []
    for core in range(NCORES):
        b = core // 2
        h = core % 2
        xb = x_full[b].astype(f8np)
        # rotate so this core's query-half occupies chunks 0..3: the program
        # uses x_t[0..3] as xq and attention is key-permutation invariant
        xr = np.ascontiguousarray(
            np.concatenate([xb[:, h * SQ:], xb[:, :h * SQ]], axis=1))
        in_maps.append({
            "x": xr,
            "xt": np.ascontiguousarray(xr.T),
            "ht": ht, "w2t": w2t, "wt": wt_t, "bop": bop_t,
        })
    return in_maps


def assemble_output(results):
    """results: list of 8 dicts with 'y' [C, SQ] fp32 -> (B, C, H, W)."""
    y = np.empty((B, C, S), dtype=np.float32)
    for core in range(NCORES):
        b = core // 2
        h = core % 2
        y[b][:, h * SQ:(h + 1) * SQ] = results[core]["y"]
    return y.reshape(B, C, H, W)


def kernel(**inputs):
    import sys
    for p in ("/opt/trn_rl_repo", "/opt/trn_rl_repo/concourse"):
        if p not in sys.path:
            sys.path.insert(0, p)
    from concourse.bass_utils import run_bass_kernel_spmd

    inputs = {k: np.asarray(v) for k, v in inputs.items()}
    nc = build_bass()
    in_maps = make_in_maps(**inputs)
    res = run_bass_kernel_spmd(nc, in_maps, core_ids=list(range(NCORES)))
    return assemble_output(res.results)


if __name__ == "__main__":
    pass


# revision 12
# speedup vs baseline: 1.3566x; 1.3566x over previous
"""Trainium2 Bass kernel for nn_CrossAttention3D (B=4, C=D=512, H=W=64).

Strategy
--------
reference:  x=(b,c,s) with s=h*w=4096;  Q/K/V = per-pixel linear (1x1 conv),
            sim = Q K^T * D^-0.5, attn = softmax(sim), o = attn V,
            y = o Wo^T + bo.

Sharding: 8 cores = (batch b in 0..3) x (query-half in 0..1); each core does
attention + output for its 2048 query tokens. No collectives.

Algebraic refactor (host folds weight-weight products, exact math):
  sim[t,s] = K_t . Q_s = x_t^T H xq_s + x_t^T wt + c_s
      H  = Wk^T (Wq*scale)   [c,c]
      wt = Wk^T (bq*scale)   [c]     (rides as the U bias)
      c_s depends only on s -> cancels in softmax.
  U = H xq + wt                      (replaces Q AND K projections)
  P = exp(x^T U)
  Z = sum_t x_t P[t,s]
  y = W2 Z / l + bo'                 (replaces V proj AND out proj)
      W2 = Wo Wv,  bo' = Wo bv + bo
      l  = ones^T P

FP8 (e4m3) + DoubleRow: every matmul contracts TWO 128-deep k-chunks per
instruction (lhsT [128,2,M], rhs [128,2,N]) at 0.5 cycles/row -- 2-4x the
bf16 rate.  Probe-validated scale factors keep all fp8 operands in e4m3's
normal range:
  x8  = fp8(x)                       (std 1.0)
  ht  = fp8(256*H^T), U psum = 256*(H xq); u8 = fp8(psum/8 + 32*wt) = 32*U
  sim psum = 32*sim;  P8 = exp(psum/32)   (ACT scale)
  z8  = fp8(Z/16),  w2t = fp8(16*W2^T) -> py = W2 Z exactly
Emulated end-to-end rel err 9.3e-3 (gate 2e-2; bf16 version was 5.8e-4).

The sim/Z loop is software-pipelined at t-chunk-PAIR granularity (sim runs
AHEAD of Z by 2 pairs, crossing query-tile boundaries).  The denominator l
accumulates as ONE bf16 DVE add per pair ([128,1024], 2x DVE rate); the last
pair joins the cross-partition ones-matmul reduction directly so the PE never
waits on the DVE tail.  U bias + Z evacuation ride on DVE tensor_scalar,
keeping ACT free for the 64 exp instructions that are its floor.
"""

import numpy as np
import ml_dtypes

bf16 = ml_dtypes.bfloat16
f8np = ml_dtypes.float8_e4m3

# Problem constants (hardcoded per harness contract)
B, C, H, W = 4, 512, 64, 64
D = 512
S = H * W          # 4096 tokens per batch
NCORES = 8
SQ = S * B // NCORES  # 2048 query tokens per core
P = 128            # partitions
NC_C = C // P      # 4 c-chunks
NT = S // P        # 32 t-chunks (keys)
NPAIR = NT // 2    # 16 t-chunk pairs
NSQ = SQ // 512    # 4 query tiles of 512
NTT = S // 512     # 8 t-tiles of 512
AHEAD = 4          # sim-ahead-of-Z pipeline depth, in pairs
NWARM = 12         # PE warm-up matmuls during the DMA head

GH = 256.0         # host scale on H (fp8 range)
GU = 32.0          # scale on U in fp8
GW = 16.0          # host scale on W2
GZI = 1.0 / 16.0   # scale applied to Z at evacuation (cancels GW)


def build_bass():
    """Build the single-core SPMD Bass program."""
    import concourse.mybir as mybir
    import concourse.tile as tile
    from concourse import bacc

    fp32 = mybir.dt.float32
    bfl = mybir.dt.bfloat16
    f8 = mybir.dt.float8e4
    AF = mybir.ActivationFunctionType
    DR = mybir.MatmulPerfMode.DoubleRow
    MUL = mybir.AluOpType.mult
    ADD = mybir.AluOpType.add

    nc = bacc.Bacc("TRN2", target_bir_lowering=False)

    # x is shipped per-core with the core's own query-half chunks FIRST
    # (chunk order rotated on host), so x_t[0..3] double as xq tiles.
    x_d = nc.dram_tensor("x", (C, S), f8, kind="ExternalInput")
    xt_d = nc.dram_tensor("xt", (S, C), f8, kind="ExternalInput")
    ht_d = nc.dram_tensor("ht", (C, C), f8, kind="ExternalInput")
    w2t_d = nc.dram_tensor("w2t", (C, C), f8, kind="ExternalInput")
    wt_d = nc.dram_tensor("wt", (P, NC_C), fp32, kind="ExternalInput")
    bop_d = nc.dram_tensor("bop", (P, NC_C), fp32, kind="ExternalInput")
    y_d = nc.dram_tensor("y", (C, SQ), fp32, kind="ExternalOutput")

    with tile.TileContext(nc) as tc:
        with (
            tc.tile_pool(name="const", bufs=1) as const,
            tc.tile_pool(name="pt", bufs=6) as ptp,
            tc.tile_pool(name="zsb", bufs=2) as zsb,
            tc.tile_pool(name="ysb", bufs=3) as ysb,
            tc.tile_pool(name="small", bufs=2) as small,
            tc.tile_pool(name="ps", bufs=4, space="PSUM") as ps,
            tc.tile_pool(name="pso", bufs=1, space="PSUM") as pso,
        ):
            # ---- PE warm-up burst: no input deps, runs while DMAs land ----
            wtile = const.tile([P, 2, 512], f8)
            nc.vector.memset(wtile, 0.01)
            wps = ps.tile([P, 512], fp32, tag="ps")
            for i in range(NWARM):
                nc.tensor.matmul(wps, wtile[:, :, 0:P], wtile,
                                 start=(i == 0), stop=(i == NWARM - 1),
                                 perf_mode=DR)
            wdump = small.tile([P, 16], fp32, tag="wdump")
            nc.vector.tensor_copy(wdump, wps[:, 0:16])

            # ---- loads, ordered + chunked so U-projection starts ASAP ----
            ht_sb = const.tile([P, NC_C, C], f8)
            nc.sync.dma_start(ht_sb, ht_d[:].rearrange("(o p) c -> p o c", p=P))
            wt_sb = const.tile([P, NC_C], fp32)
            nc.sync.dma_start(wt_sb, wt_d[:])
            x_t = [const.tile([P, NC_C, 512], f8, tag=f"x{tt}", name=f"x{tt}")
                   for tt in range(NTT)]
            xt_sb = const.tile([P, NT, C], f8)

            def load_x(tt):
                nc.sync.dma_start(
                    x_t[tt], x_d[:, tt * 512:(tt + 1) * 512]
                    .rearrange("(o p) s -> p o s", p=P))

            def load_xt(i):
                nc.sync.dma_start(
                    xt_sb[:, i * 8:(i + 1) * 8, :],
                    xt_d[i * 1024:(i + 1) * 1024, :]
                    .rearrange("(o p) c -> p o c", p=P))

            load_x(0); load_x(1); load_xt(0)
            load_x(2); load_x(3); load_xt(1)
            load_x(4); load_x(5); load_xt(2)
            load_x(6); load_x(7); load_xt(3)
            w2t_sb = const.tile([P, NC_C, C], f8)
            nc.sync.dma_start(w2t_sb, w2t_d[:].rearrange("(o p) c -> p o c", p=P))
            bop_sb = const.tile([P, NC_C], fp32)
            nc.sync.dma_start(bop_sb, bop_d[:])
            ones_bf = const.tile([P, 1], bfl)
            nc.vector.memset(ones_bf, 1.0)
            ones8 = const.tile([P, 1], f8)
            nc.vector.memset(ones8, 1.0)

            u_sb = const.tile([P, NC_C, SQ], f8)   # u8[c, sq] = 32*U

            # ---- U projection: u8 = (256*H @ xq)/8 + 32*wt  (DVE epilogue) ----
            for st in range(NSQ):
                for co in range(NC_C):
                    pu = ps.tile([P, 512], fp32, tag="ps")
                    for j in range(2):
                        nc.tensor.matmul(
                            pu,
                            ht_sb[:, 2 * j:2 * j + 2, co * P:(co + 1) * P],
                            x_t[st][:, 2 * j:2 * j + 2, :],
                            start=(j == 0), stop=(j == 1), perf_mode=DR,
                        )
                    nc.vector.tensor_scalar(
                        u_sb[:, co, st * 512:(st + 1) * 512], pu,
                        1.0 / 8.0, wt_sb[:, co:co + 1], MUL, ADD)

            # ---- attention: flat software pipeline over (st, pair) units ----
            units = [(st, pj) for st in range(NSQ) for pj in range(NPAIR)]
            total = len(units)
            pts = [None] * total
            state = {}  # per-st live tiles: po, l_acc

            def sim_step(i):
                st, pj = units[i]
                pt = ptp.tile([P, 2, 512], f8, tag="pt")
                for k in range(2):
                    tch = 2 * pj + k
                    pss = ps.tile([P, 512], fp32, tag="ps")
                    for j in range(2):
                        nc.tensor.matmul(
                            pss,
                            x_t[tch // 4][:, 2 * j:2 * j + 2,
                                          (tch % 4) * P:(tch % 4 + 1) * P],
                            u_sb[:, 2 * j:2 * j + 2,
                                 st * 512:(st + 1) * 512],
                            start=(j == 0), stop=(j == 1), perf_mode=DR,
                        )
                    nc.scalar.activation(pt[:, k, :], pss, AF.Exp,
                                         scale=1.0 / GU)
                pts[i] = pt

            for i in range(AHEAD):
                sim_step(i)
            for i, (st, pj) in enumerate(units):
                if i + AHEAD < total and pj != NPAIR - 1:
                    sim_step(i + AHEAD)
                pt = pts[i]
                if pj == 0:
                    state["po"] = pso.tile([P, NC_C, 512], fp32, tag="po",
                                           name="po")
                    state["lacc"] = small.tile([P, 2, 512], bfl, tag="lacc",
                                               name="lacc")
                po, l_acc = state["po"], state["lacc"]
                # Z[c, sq] += xt[t-pair, c-chunk]^T P  (fp8 DoubleRow)
                for cc in range(NC_C):
                    nc.tensor.matmul(
                        po[:, cc, :],
                        xt_sb[:, 2 * pj:2 * pj + 2, cc * P:(cc + 1) * P],
                        pt,
                        start=(pj == 0), stop=(pj == NPAIR - 1), perf_mode=DR,
                    )
                if pj == 0:
                    nc.vector.tensor_copy(l_acc, pt)
                elif pj < NPAIR - 1:
                    # last pair skips the chain; it joins the denominator
                    # directly in PSUM via two fp8 matmuls so the PE never
                    # waits on the accumulator tail
                    nc.vector.tensor_add(out=l_acc, in0=l_acc, in1=pt)
                    pts[i] = None

                if pj == NPAIR - 1:
                    # ---- epilogue for query tile st ----
                    # emitted BEFORE the look-ahead sim_step so the z
                    # evacuations sit ahead of the next exps in the ACT queue
                    sq_sl = slice(st * 512, (st + 1) * 512)
                    z_t = zsb.tile([P, NC_C, 512], f8, tag="z")
                    nc.vector.tensor_scalar_mul(z_t[:, 0, :], po[:, 0, :], GZI)
                    nc.scalar.activation(z_t[:, 1, :], po[:, 1, :],
                                         AF.Identity, scale=GZI)

                    plt = ps.tile([P, 512], fp32, tag="ps")
                    pl = plt[0:1, :]
                    nc.tensor.matmul(pl, ones_bf, l_acc[:, 0, :],
                                     start=True, stop=False)
                    nc.tensor.matmul(pl, ones_bf, l_acc[:, 1, :],
                                     start=False, stop=False)
                    nc.tensor.matmul(pl, ones8, pt[:, 0, :],
                                     start=False, stop=False)
                    nc.tensor.matmul(pl, ones8, pt[:, 1, :],
                                     start=False, stop=True)
                    pts[i] = None
                    rl = small.tile([1, 512], fp32, tag="rl")
                    nc.vector.reciprocal_approx_fast(rl, pl)
                    rlb = small.tile([P, 512], fp32, tag="rlb")
                    nc.gpsimd.partition_broadcast(rlb, rl)
                    nc.vector.tensor_scalar_mul(z_t[:, 2, :], po[:, 2, :], GZI)
                    nc.scalar.activation(z_t[:, 3, :], po[:, 3, :],
                                         AF.Identity, scale=GZI)

                    # y[c, sq] = (W2 Z) * rl + bo'.  py chunks live in the ps
                    # pool (NOT the po slot), so po(st+1) is free as soon as
                    # the four z evacuations have drained it -- Z(st+1) never
                    # waits on the rl/ytmp chain.
                    pys = []
                    for co in range(NC_C):
                        py_c = ps.tile([P, 512], fp32, tag="ps")
                        for j in range(2):
                            nc.tensor.matmul(
                                py_c,
                                w2t_sb[:, 2 * j:2 * j + 2,
                                       co * P:(co + 1) * P],
                                z_t[:, 2 * j:2 * j + 2, :],
                                start=(j == 0), stop=(j == 1), perf_mode=DR,
                            )
                        pys.append(py_c)
                    if i + AHEAD < total:
                        sim_step(i + AHEAD)
                    # py -> y: GpSimd cannot read PSUM, so odd chunks drain
                    # via an ACT Identity copy to SBUF (then GpSimd applies
                    # rl); even chunks are direct DVE muls from PSUM.  Two
                    # engines drain the py bank rotation in parallel.
                    for co in range(NC_C):
                        ytmp = ysb.tile([P, 512], fp32, tag="ytmp")
                        if co % 2 == 0:
                            nc.vector.tensor_mul(out=ytmp, in0=pys[co],
                                                 in1=rlb)
                        else:
                            yc = ysb.tile([P, 512], fp32, tag="yc")
                            nc.scalar.activation(yc, pys[co], AF.Identity)
                            nc.gpsimd.tensor_mul(out=ytmp, in0=yc, in1=rlb)
                        yt = ysb.tile([P, 512], fp32, tag="y")
                        nc.scalar.activation(yt, ytmp, AF.Identity,
                                             bias=bop_sb[:, co:co + 1])
                        nc.sync.dma_start(y_d[co * P:(co + 1) * P, sq_sl], yt)

    nc.finalize()
    return nc


def make_in_maps(q, Wq, bq, Wk, bk, Wv, bv, Wo, bo):
    """Host-side sharding + weight folding. Returns list of 8 input dicts."""
    scale = float(D) ** -0.5
    x_full = np.ascontiguousarray(q.reshape(B, C, S)).astype(np.float32)

    Hm = Wk.T.astype(np.float32) @ (Wq.astype(np.float32) * scale)   # [c, c]
    wt = Wk.T.astype(np.float32) @ (bq.astype(np.float32) * scale)   # [c]
    W2 = Wo.astype(np.float32) @ Wv.astype(np.float32)               # [c, c]
    bop = Wo.astype(np.float32) @ bv.astype(np.float32) + bo         # [c]

    ht = np.ascontiguousarray(Hm.T * GH).astype(f8np)
    w2t = np.ascontiguousarray(W2.T * GW).astype(f8np)
    wt_t = np.ascontiguousarray((GU * wt).reshape(NC_C, P).T).astype(np.float32)
    bop_t = np.ascontiguousarray(bop.reshape(NC_C, P).T).astype(np.float32)

    in_maps = []
    for core in range(NCORES):
        b = core // 2
        h = core % 2
        xb = x_full[b].astype(f8np)
        # rotate so this core's query-half occupies chunks 0..3: the program
        # uses x_t[0..3] as xq and attention is key-permutation invariant
        xr = np.ascontiguousarray(
            np.concatenate([xb[:, h * SQ:], xb[:, :h * SQ]], axis=1))
        in_maps.append({
            "x": xr,
            "xt": np.ascontiguousarray(xr.T),
            "ht": ht, "w2t": w2t, "wt": wt_t, "bop": bop_t,
        })
    return in_maps


def assemble_output(results):
    """results: list of 8 dicts with 'y' [C, SQ] fp32 -> (B, C, H, W)."""
    y = np.empty((B, C, S), dtype=np.float32)
    for core in range(NCORES):
        b = core // 2
        h = core % 2
        y[b][:, h * SQ:(h + 1) * SQ] = results[core]["y"]
    return y.reshape(B, C, H, W)


def kernel(**inputs):
    import sys
    for p in ("/opt/trn_rl_repo", "/opt/trn_rl_repo/concourse"):
        if p not in sys.path:
            sys.path.insert(0, p)
    from concourse.bass_utils import run_bass_kernel_spmd

    inputs = {k: np.asarray(v) for k, v in inputs.items()}
    nc = build_bass()
    in_maps = make_in_maps(**inputs)
    res = run_bass_kernel_spmd(nc, in_maps, core_ids=list(range(NCORES)))
    return assemble_output(res.results)


if __name__ == "__main__":
    pass
